# revision 37
# baseline (speedup 1.0000x reference)
# Trainium2 Bass kernel for DeltaPredictor (conv encoder -> GRU -> deconv decoder).
#
# Layout strategy (per core, batch-parallel over 8 cores, BC=64 batch each):
#   Everything on-device runs in "transposed" space: feature/hidden dims on SBUF
#   partitions, (time, batch) on the free axis. This keeps the GRU recurrence
#   transpose-free and makes every DRAM access t-contiguous (1792B+ bursts).
#
#   - hidden permutation h' = ij*32 + c (h = c*16 + ij) makes the decoder
#     block-diagonal at 128 granularity (per-half matmuls).
#   - input-feature permutation f' = ij*32 + o makes the encoder block-diagonal.
#   - encoder prev/curr frames are fused into one K=128 matmul by storing a
#     time-shifted copy of the pixels in partitions 64:128 of each pixcat tile.
#   Both permutations are absorbed into host-side weight reshuffles.
#
#   GRU step: gx (from the per-block input GEMM) is preloaded into PSUM by the
#   Scalar engine, the 48 recurrent matmuls accumulate on top (start=False),
#   and sigmoid/tanh read PSUM directly. The n-gate is split in two halves so
#   the post-matmul serial tail is half as long.

import numpy as np

CH = 32
HID = 512
NCORES = 8
BLK = 7

_cache = {}

# test instrumentation (harness uses defaults): set TRACE=True before calling
# kernel() to capture an NTFF profile; the result lands in LAST_RESULT
TRACE = False
LAST_RESULT = None


def _build(T, BC):
    from contextlib import ExitStack

    import concourse.tile as tile
    from concourse import bacc, mybir

    f32 = mybir.dt.float32
    bf16 = mybir.dt.bfloat16
    AF = mybir.ActivationFunctionType
    OP = mybir.AluOpType

    T2 = T - 2
    NB = T2 // BLK
    assert NB * BLK == T2

    nc = bacc.Bacc("TRN2", target_bir_lowering=False)

    framesT = nc.dram_tensor("framesT", [2, 128, T, BC], f32, kind="ExternalInput")
    whh = nc.dram_tensor("whh", [128, 4, 12, 128], bf16, kind="ExternalInput")
    wih = nc.dram_tensor("wih", [128, 4, 12, 128], bf16, kind="ExternalInput")
    wenc = nc.dram_tensor("wenc", [128, 128], bf16, kind="ExternalInput")
    wdec = nc.dram_tensor("wdec", [128, 64], bf16, kind="ExternalInput")
    encb = nc.dram_tensor("encb", [128, 1], f32, kind="ExternalInput")
    encb2 = nc.dram_tensor("encb2", [128, 1], f32, kind="ExternalInput")
    bcomb = nc.dram_tensor("bcomb", [128, 12], f32, kind="ExternalInput")
    bhhnb = nc.dram_tensor("bhhnb", [128, 4, BC], bf16, kind="ExternalInput")
    decb = nc.dram_tensor("decb", [128, 1], f32, kind="ExternalInput")
    outT = nc.dram_tensor("outT", [2, 128, T2, BC], f32, kind="ExternalOutput")

    with tile.TileContext(nc) as tc, ExitStack() as ctx:
        consts = ctx.enter_context(tc.tile_pool(name="consts", bufs=1))
        featp = ctx.enter_context(tc.tile_pool(name="featp", bufs=2))
        gxp = ctx.enter_context(tc.tile_pool(name="gxp", bufs=2))
        # bufs=3: the decoder of block B reads outsbuf while block B+2's steps
        # need a fresh slot — a third buffer removes that WAR stall window
        outsp = ctx.enter_context(tc.tile_pool(name="outsp", bufs=3))
        stepp = ctx.enter_context(tc.tile_pool(name="stepp", bufs=3))
        decp = ctx.enter_context(tc.tile_pool(name="decp", bufs=2))
        encp = ctx.enter_context(tc.tile_pool(name="encp", bufs=2))
        ps_rz = ctx.enter_context(tc.tile_pool(name="ps_rz", bufs=2, space="PSUM"))
        ps_n = ctx.enter_context(tc.tile_pool(name="ps_n", bufs=2, space="PSUM"))
        ps_gx = ctx.enter_context(tc.tile_pool(name="ps_gx", bufs=2, space="PSUM"))
        # enc and dec share one pool: they run at different points of the
        # block so bank pressure (8 banks total) stays within budget
        ps_ed = ctx.enter_context(tc.tile_pool(name="ps_ed", bufs=2, space="PSUM"))

        # ramp order: encoder weights/biases first (enc(0) is the first
        # dependent work), the big GRU weight tiles last
        wenc_sb = consts.tile([128, 128], bf16)
        nc.sync.dma_start(out=wenc_sb[:], in_=wenc[:])
        encb_sb = consts.tile([128, 1], f32)
        nc.sync.dma_start(out=encb_sb[:], in_=encb[:])
        encb2_sb = consts.tile([128, 1], f32)
        nc.sync.dma_start(out=encb2_sb[:], in_=encb2[:])
        bcomb_sb = consts.tile([128, 12], f32)
        nc.sync.dma_start(out=bcomb_sb[:], in_=bcomb[:])
        bhhnb_sb = consts.tile([128, 4, BC], bf16)
        nc.sync.dma_start(out=bhhnb_sb[:], in_=bhhnb[:])
        decb_sb = consts.tile([128, 1], f32)
        nc.sync.dma_start(out=decb_sb[:], in_=decb[:])
        wdec_sb = consts.tile([128, 64], bf16)
        nc.sync.dma_start(out=wdec_sb[:], in_=wdec[:])
        wih_sb = consts.tile([128, 4, 12, 128], bf16)
        nc.sync.dma_start(out=wih_sb[:], in_=wih[:])
        whh_sb = consts.tile([128, 4, 12, 128], bf16)
        nc.sync.dma_start(out=whh_sb[:], in_=whh[:])

        # pixcat[i]: partitions 0:64 = pixel rows [64i, 64i+64) at time t,
        # partitions 64:128 = same rows at t+1 (cast to bf16 by the DMA).
        # The +1 shift bakes the curr-frame into the same K=128 matmul as prev.
        pixcat = []
        for i in range(4):
            pt = consts.tile([128, T, BC], bf16, name=f"pixcat{i}")
            pixcat.append(pt)
        nstrip = 8
        ts_ = T // nstrip
        for s in range(nstrip):
            t0 = s * ts_
            for i in range(4):
                tilei, base = i // 2, 64 * (i % 2)
                nc.gpsimd.dma_start(
                    out=pixcat[i][0:64, t0 : t0 + ts_, :],
                    in_=framesT[tilei, base : base + 64, t0 : t0 + ts_, :],
                )
                # shifted copy; last strip is one step short (frame T-1 has no
                # successor and t=T-1 of the curr-half is never read)
                te = min(t0 + ts_, T - 1)
                nc.gpsimd.dma_start(
                    out=pixcat[i][64:128, t0:te, :],
                    in_=framesT[tilei, base : base + 64, t0 + 1 : te + 1, :],
                )

        h0bf = consts.tile([128, 4, BC], bf16)
        nc.vector.memset(h0bf[:], 0.0)

        def enc_unit(beta, featbuf, i):
            pse = ps_ed.tile([128, BLK, BC], f32, name="ped")
            t0 = BLK * beta
            nc.tensor.matmul(
                out=pse[:],
                lhsT=wenc_sb[:],
                rhs=pixcat[i][:, t0 : t0 + BLK, :],
                start=True,
                stop=True,
            )
            # exact GELU via Erf: feat = (1 + erf((x+b)/sqrt2)) * (x+b), the
            # 0.5 is folded into w_ih host-side. erf and the bias-add both
            # read PSUM directly and run in parallel on ScE/DVE.
            xsb = encp.tile([128, BLK, BC], f32, name="xsb")
            nc.vector.tensor_scalar_add(out=xsb[:], in0=pse[:], scalar1=encb_sb[:, 0:1])
            erft = encp.tile([128, BLK, BC], f32, name="erft")
            nc.scalar.activation(
                out=erft[:],
                in_=pse[:],
                func=AF.Erf,
                bias=encb2_sb[:, 0:1],
                scale=0.7071067811865476,
            )
            nc.vector.scalar_tensor_tensor(
                out=featbuf[:, i, :, :],
                in0=erft[:],
                scalar=1.0,
                in1=xsb[:],
                op0=OP.add,
                op1=OP.mult,
            )

        def emit_enc(beta, featbuf):
            for i in range(4):
                enc_unit(beta, featbuf, i)

        def gx_unit(featbuf, gxbuf, m):
            psg = ps_gx.tile([128, BLK, BC], f32, name="psg")
            for k in range(4):
                nc.tensor.matmul(
                    out=psg[:],
                    lhsT=wih_sb[:, k, m, :],
                    rhs=featbuf[:, k, :, :],
                    start=(k == 0),
                    stop=(k == 3),
                )
            nc.scalar.activation(
                out=gxbuf[:, m, :, :],
                in_=psg[:],
                func=AF.Identity,
                bias=bcomb_sb[:, m : m + 1],
                scale=1.0,
            )

        def emit_gx(beta, featbuf, gxbuf):
            for m in range(12):
                gx_unit(featbuf, gxbuf, m)

        def emit_step(hbf, gxbuf, tt, outsbuf):
            # PSUM preloads (off the critical path): rz gets gx, n gets b_hh_n.
            # The 48 recurrent matmuls then accumulate straight on top
            # (start=False) and the activations read finished sums from PSUM.
            prz = ps_rz.tile([128, 8, BC], f32, name="prz")
            nc.scalar.activation(
                out=prz[:], in_=gxbuf[:, 0:8, tt, :], func=AF.Identity
            )
            pn = ps_n.tile([128, 4, BC], f32, name="pn")
            nc.vector.tensor_copy(out=pn[:], in_=bhhnb_sb[:])
            for m in range(4):
                for k in range(4):
                    nc.tensor.matmul(
                        out=prz[:, m, :],
                        lhsT=whh_sb[:, k, m, :],
                        rhs=hbf[:, k, :],
                        start=False,
                        stop=(k == 3),
                        skip_group_check=True,
                    )
            rz = stepp.tile([128, 8, BC], f32, name="rz")
            # r-sigmoid fires right after the r matmuls and hides under the
            # z/n matmul stream — it is off the step critical chain
            nc.scalar.activation(out=rz[:, 0:4, :], in_=prz[:, 0:4, :], func=AF.Sigmoid)
            for m in range(4, 8):
                for k in range(4):
                    nc.tensor.matmul(
                        out=prz[:, m, :],
                        lhsT=whh_sb[:, k, m, :],
                        rhs=hbf[:, k, :],
                        start=False,
                        stop=(k == 3),
                        skip_group_check=True,
                    )
            nc.scalar.activation(out=rz[:, 4:8, :], in_=prz[:, 4:8, :], func=AF.Sigmoid)
            # w = z*h and v = 1-z run on GpSimd (SBUF-only ops) under the
            # n-gate matmuls, keeping DVE free for the serial tail
            v = stepp.tile([128, 4, BC], f32, name="v")
            nc.gpsimd.tensor_scalar(
                out=v[:], in0=rz[:, 4:8, :], scalar1=-1.0, scalar2=1.0,
                op0=OP.mult, op1=OP.add,
            )
            w = stepp.tile([128, 4, BC], f32, name="w")
            nc.gpsimd.tensor_mul(out=w[:], in0=rz[:, 4:8, :], in1=hbf[:])
            for m in range(8, 12):
                for k in range(4):
                    nc.tensor.matmul(
                        out=pn[:, m - 8, :],
                        lhsT=whh_sb[:, k, m, :],
                        rhs=hbf[:, k, :],
                        start=False,
                        stop=(k == 3),
                        skip_group_check=True,
                    )
            # n = tanh(gx_n + r*(gh_n + b_hh_n)); pn already holds gh_n+b_hh_n.
            # Split in half so the serial tail after the last matmul is short:
            # the a-half runs while the b-half matmuls finish.
            for h2, (c0, c1) in enumerate(((0, 2), (2, 4))):
                t2 = stepp.tile([128, 2, BC], f32, name=f"t2{h2}")
                nc.vector.tensor_mul(
                    out=t2[:], in0=rz[:, c0:c1, :], in1=pn[:, c0:c1, :]
                )
                npre = stepp.tile([128, 2, BC], f32, name=f"npre{h2}")
                nc.vector.tensor_add(
                    out=npre[:], in0=t2[:], in1=gxbuf[:, 8 + c0 : 8 + c1, tt, :]
                )
                nsb = stepp.tile([128, 2, BC], f32, name=f"nsb{h2}")
                nc.scalar.activation(out=nsb[:], in_=npre[:], func=AF.Tanh)
                u = stepp.tile([128, 2, BC], f32, name=f"u{h2}")
                nc.vector.tensor_mul(out=u[:], in0=nsb[:], in1=v[:, c0:c1, :])
                nc.vector.tensor_add(
                    out=outsbuf[:, c0:c1, tt, :], in0=u[:], in1=w[:, c0:c1, :]
                )

        def dec_unit(beta, outsbuf, i2, currt):
            psd = ps_ed.tile([128, BLK, BC], f32, name="ped")
            for half in range(2):
                i = i2 * 2 + half
                nc.tensor.matmul(
                    out=psd[64 * half : 64 * half + 64, :, :],
                    lhsT=wdec_sb[:],
                    rhs=outsbuf[:, i, :, :],
                    start=True,
                    stop=True,
                )
            delta = decp.tile([128, BLK, BC], f32, name="delta")
            nc.scalar.activation(
                out=delta[:], in_=psd[:], func=AF.Tanh, bias=decb_sb[:, 0:1]
            )
            pred = decp.tile([128, BLK, BC], f32, name="pred")
            nc.vector.tensor_add(out=pred[:], in0=delta[:], in1=currt[:])
            nc.vector.tensor_scalar(
                out=pred[:],
                in0=pred[:],
                scalar1=0.0,
                scalar2=1.0,
                op0=OP.max,
                op1=OP.min,
            )
            nc.sync.dma_start(
                out=outT[i2, :, BLK * beta : BLK * beta + BLK, :],
                in_=pred[:],
            )

        def emit_pipeline():
            featbuf = featp.tile([128, 4, BLK, BC], bf16, name="featbuf")
            emit_enc(0, featbuf)
            gxbuf = gxp.tile([128, 12, BLK, BC], bf16, name="gxbuf")
            emit_gx(0, featbuf, gxbuf)

            hbf = h0bf[:]
            pending = []
            for beta in range(NB):
                cur_gx = gxbuf
                currts = []
                for i2 in range(2):
                    currt = decp.tile([128, BLK, BC], f32, name=f"curr{i2}")
                    nc.sync.dma_start(
                        out=currt[:],
                        in_=framesT[
                            i2, :, BLK * beta + 1 : BLK * beta + 1 + BLK, :
                        ],
                    )
                    currts.append(currt)
                # interleave the next block's enc/gx units (and the previous
                # block's decoder) between steps, ~3 units per step, instead
                # of letting 18 units pile up at the block boundary
                if beta + 1 < NB:
                    featbuf = featp.tile([128, 4, BLK, BC], bf16, name="featbuf")
                    gxbuf = gxp.tile([128, 12, BLK, BC], bf16, name="gxbuf")
                    fb, gb = featbuf, gxbuf
                    pending.extend(
                        [
                            lambda i=i, f=fb, b=beta + 1: enc_unit(b, f, i)
                            for i in range(4)
                        ]
                        + [lambda m=m, f=fb, g=gb: gx_unit(f, g, m) for m in range(12)]
                    )
                outsbuf = outsp.tile([128, 4, BLK, BC], bf16, name="outsbuf")
                for tt in range(BLK):
                    emit_step(hbf, cur_gx, tt, outsbuf)
                    hbf = outsbuf[:, :, tt, :]
                    for _ in range(3):
                        if pending:
                            pending.pop(0)()
                ob = outsbuf
                pending.extend(
                    [
                        lambda i2=i2, o=ob, c=currts[i2], b=beta: dec_unit(b, o, i2, c)
                        for i2 in range(2)
                    ]
                )
            while pending:
                pending.pop(0)()

        emit_pipeline()

    nc.compile()
    return nc


def _prep_weights(conv_w, conv_b, w_ih, w_hh, b_ih, b_hh, deconv_w, deconv_b, BC):
    """Host-side weight reshuffles into the kernel's permuted/tiled layouts."""
    import ml_dtypes

    bf = ml_dtypes.bfloat16

    idx = np.arange(HID)
    hmap = (idx % 32) * 16 + (idx // 32)  # h' -> h  (h' = ij*32 + c)

    # 0.5 from the erf-form GELU is folded into w_ih (feat' = 2*gelu(x))
    w_ih2 = 0.5 * w_ih.reshape(3, HID, HID)[:, hmap, :][:, :, hmap].reshape(
        3 * HID, HID
    )
    w_hh2 = w_hh.reshape(3, HID, HID)[:, hmap, :][:, :, hmap].reshape(3 * HID, HID)
    b_ih2 = b_ih.reshape(3, HID)[:, hmap].reshape(3 * HID)
    b_hh2 = b_hh.reshape(3, HID)[:, hmap].reshape(3 * HID)

    # (kk, k, m, mm): lhsT(k,m)[kk,mm] = W2[m*128+mm, k*128+kk]
    whh_t = np.ascontiguousarray(
        w_hh2.T.reshape(4, 128, 12, 128).transpose(1, 0, 2, 3)
    ).astype(bf)
    wih_t = np.ascontiguousarray(
        w_ih2.T.reshape(4, 128, 12, 128).transpose(1, 0, 2, 3)
    ).astype(bf)

    # encoder: rows u=16p+4j+q in [0,64) for the prev frame (c=1), rows
    # 64+u for the curr frame (c=0, via the pixcat +1 time shift);
    # cols j2*32+o. The block is identical for every patch-row i.
    wenc_h = np.zeros((128, 128), np.float32)
    u = np.arange(64)
    p, j, q = (u >> 4) & 3, (u >> 2) & 3, u & 3
    for s, c in ((0, 1), (1, 0)):
        blockw = np.zeros((64, 128), np.float32)
        for j2 in range(4):
            mask = j == j2
            blockw[mask, j2 * 32 : j2 * 32 + 32] = conv_w[:, c, p[mask], q[mask]].T
        wenc_h[64 * s : 64 * s + 64, :] = blockw
    wenc_t = wenc_h.astype(bf)

    # decoder: rows j*32+c, cols u2 = p*16 + j2*4 + q
    wdec_h = np.zeros((128, 64), np.float32)
    for jj in range(4):
        for c in range(CH):
            for pp in range(4):
                for qq in range(4):
                    wdec_h[jj * 32 + c, pp * 16 + jj * 4 + qq] = deconv_w[c, 0, pp, qq]
    wdec_t = wdec_h.astype(bf)

    encb_h = np.zeros((128, 1), np.float32)
    encb_h[:, 0] = conv_b[np.arange(128) % 32]
    encb2_h = encb_h * 0.7071067811865476

    bcomb_h = np.zeros((128, 12), np.float32)
    for m in range(12):
        g = m * 128 + np.arange(128)
        bcomb_h[:, m] = b_ih2[g] + (b_hh2[g] if m < 8 else 0.0)

    bhhnb_h = np.zeros((128, 4, BC), np.float32)
    for c in range(4):
        bhhnb_h[:, c, :] = b_hh2[1024 + c * 128 + np.arange(128)][:, None]
    bhhnb_h = bhhnb_h.astype(bf)

    decb_h = np.full((128, 1), float(deconv_b[0]), np.float32)

    return dict(
        whh=whh_t,
        wih=wih_t,
        wenc=wenc_t,
        wdec=wdec_t,
        encb=encb_h,
        encb2=encb2_h,
        bcomb=bcomb_h,
        bhhnb=bhhnb_h,
        decb=decb_h,
    )


def kernel(frames, conv_w, conv_b, w_ih, w_hh, b_ih, b_hh, deconv_w, deconv_b):
    from concourse.bass_utils import run_bass_kernel_spmd

    frames = np.asarray(frames, np.float32)
    conv_w = np.asarray(conv_w, np.float32)
    conv_b = np.asarray(conv_b, np.float32)
    w_ih = np.asarray(w_ih, np.float32)
    w_hh = np.asarray(w_hh, np.float32)
    b_ih = np.asarray(b_ih, np.float32)
    b_hh = np.asarray(b_hh, np.float32)
    deconv_w = np.asarray(deconv_w, np.float32)
    deconv_b = np.asarray(deconv_b, np.float32)

    B, T = frames.shape[0], frames.shape[1]
    T2 = T - 2
    BC = B // NCORES

    key = (T, BC)
    if key not in _cache:
        _cache[key] = _build(T, BC)
    nc = _cache[key]

    wmap = _prep_weights(
        conv_w, conv_b, w_ih, w_hh, b_ih, b_hh, deconv_w, deconv_b, BC
    )

    fr = frames.reshape(B, T, 256)
    in_maps = []
    for c in range(NCORES):
        sl = fr[c * BC : (c + 1) * BC]  # (BC, T, 256)
        framesT_c = np.ascontiguousarray(sl.transpose(2, 1, 0)).reshape(
            2, 128, T, BC
        )
        m = dict(wmap)
        m["framesT"] = framesT_c
        in_maps.append(m)

    res = run_bass_kernel_spmd(
        nc, in_maps, core_ids=list(range(NCORES)), trace=TRACE
    )
    global LAST_RESULT
    LAST_RESULT = res

    out = np.empty((B, T2, 1, 16, 16), np.float32)
    for c in range(NCORES):
        o = res.results[c]["outT"]  # (2, 128, T2, BC)
        o = o.reshape(256, T2, BC).transpose(2, 1, 0).reshape(BC, T2, 1, 16, 16)
        out[c * BC : (c + 1) * BC] = o
    return out


# revision 38
# speedup vs baseline: 1.1841x; 1.1841x over previous
# Trainium2 Bass kernel for DeltaPredictor (conv encoder -> GRU -> deconv decoder).
#
# Layout strategy (per core, batch-parallel over 8 cores, BC=64 batch each):
#   Everything on-device runs in "transposed" space: feature/hidden dims on SBUF
#   partitions, (time, batch) on the free axis. This keeps the GRU recurrence
#   transpose-free and makes every DRAM access t-contiguous (1792B+ bursts).
#
#   - hidden permutation h' = ij*32 + c (h = c*16 + ij) makes the decoder
#     block-diagonal at 128 granularity (per-half matmuls).
#   - input-feature permutation f' = ij*32 + o makes the encoder block-diagonal.
#   - encoder prev/curr frames are fused into one K=128 matmul by storing a
#     time-shifted copy of the pixels in partitions 64:128 of each pixcat tile.
#   Both permutations are absorbed into host-side weight reshuffles.
#
#   GRU step: gx (from the per-block input GEMM) is preloaded into PSUM by the
#   Scalar engine, the 48 recurrent matmuls accumulate on top (start=False),
#   and sigmoid/tanh read PSUM directly. The n-gate is split in two halves so
#   the post-matmul serial tail is half as long.

import numpy as np

CH = 32
HID = 512
NCORES = 8
BLK = 7

_cache = {}

# test instrumentation (harness uses defaults): set TRACE=True before calling
# kernel() to capture an NTFF profile; the result lands in LAST_RESULT
TRACE = False
LAST_RESULT = None


def _build(T, BC):
    from contextlib import ExitStack

    import concourse.tile as tile
    from concourse import bacc, mybir

    f32 = mybir.dt.float32
    bf16 = mybir.dt.bfloat16
    AF = mybir.ActivationFunctionType
    OP = mybir.AluOpType

    T2 = T - 2
    NB = T2 // BLK
    assert NB * BLK == T2

    nc = bacc.Bacc("TRN2", target_bir_lowering=False)

    framesT = nc.dram_tensor("framesT", [2, 128, T, BC], f32, kind="ExternalInput")
    whh = nc.dram_tensor("whh", [128, 4, 12, 128], bf16, kind="ExternalInput")
    wih = nc.dram_tensor("wih", [128, 4, 12, 128], bf16, kind="ExternalInput")
    wenc = nc.dram_tensor("wenc", [128, 128], bf16, kind="ExternalInput")
    wdec = nc.dram_tensor("wdec", [128, 64], bf16, kind="ExternalInput")
    encb = nc.dram_tensor("encb", [128, 1], f32, kind="ExternalInput")
    encb2 = nc.dram_tensor("encb2", [128, 1], f32, kind="ExternalInput")
    bcomb = nc.dram_tensor("bcomb", [128, 12], f32, kind="ExternalInput")
    bhhnb = nc.dram_tensor("bhhnb", [128, 4, BC], f32, kind="ExternalInput")
    decb = nc.dram_tensor("decb", [128, 1], f32, kind="ExternalInput")
    outT = nc.dram_tensor("outT", [2, 128, T2, BC], f32, kind="ExternalOutput")

    with tile.TileContext(nc) as tc, ExitStack() as ctx:
        consts = ctx.enter_context(tc.tile_pool(name="consts", bufs=1))
        featp = ctx.enter_context(tc.tile_pool(name="featp", bufs=2))
        gxp = ctx.enter_context(tc.tile_pool(name="gxp", bufs=2))
        # bufs=3: the decoder of block B reads outsbuf while block B+2's steps
        # need a fresh slot — a third buffer removes that WAR stall window
        outsp = ctx.enter_context(tc.tile_pool(name="outsp", bufs=3))
        stepp = ctx.enter_context(tc.tile_pool(name="stepp", bufs=3))
        decp = ctx.enter_context(tc.tile_pool(name="decp", bufs=2))
        encp = ctx.enter_context(tc.tile_pool(name="encp", bufs=2))
        ps_rz = ctx.enter_context(tc.tile_pool(name="ps_rz", bufs=2, space="PSUM"))
        ps_n = ctx.enter_context(tc.tile_pool(name="ps_n", bufs=2, space="PSUM"))
        ps_gx = ctx.enter_context(tc.tile_pool(name="ps_gx", bufs=2, space="PSUM"))
        # enc and dec share one pool: they run at different points of the
        # block so bank pressure (8 banks total) stays within budget
        ps_ed = ctx.enter_context(tc.tile_pool(name="ps_ed", bufs=2, space="PSUM"))

        whh_sb = consts.tile([128, 4, 12, 128], bf16)
        nc.sync.dma_start(out=whh_sb[:], in_=whh[:])
        wih_sb = consts.tile([128, 4, 12, 128], bf16)
        nc.sync.dma_start(out=wih_sb[:], in_=wih[:])
        wenc_sb = consts.tile([128, 128], bf16)
        nc.sync.dma_start(out=wenc_sb[:], in_=wenc[:])
        wdec_sb = consts.tile([128, 64], bf16)
        nc.sync.dma_start(out=wdec_sb[:], in_=wdec[:])
        encb_sb = consts.tile([128, 1], f32)
        nc.sync.dma_start(out=encb_sb[:], in_=encb[:])
        encb2_sb = consts.tile([128, 1], f32)
        nc.sync.dma_start(out=encb2_sb[:], in_=encb2[:])
        bcomb_sb = consts.tile([128, 12], f32)
        nc.sync.dma_start(out=bcomb_sb[:], in_=bcomb[:])
        bhhnb_sb = consts.tile([128, 4, BC], f32)
        nc.sync.dma_start(out=bhhnb_sb[:], in_=bhhnb[:])
        decb_sb = consts.tile([128, 1], f32)
        nc.sync.dma_start(out=decb_sb[:], in_=decb[:])

        # pixcat[i]: partitions 0:64 = pixel rows [64i, 64i+64) at time t,
        # partitions 64:128 = same rows at t+1 (cast to bf16 by the DMA).
        # The +1 shift bakes the curr-frame into the same K=128 matmul as prev.
        pixcat = []
        for i in range(4):
            pt = consts.tile([128, T, BC], bf16, name=f"pixcat{i}")
            pixcat.append(pt)
        nstrip = 4
        ts_ = T // nstrip
        for s in range(nstrip):
            t0 = s * ts_
            for i in range(4):
                tilei, base = i // 2, 64 * (i % 2)
                nc.gpsimd.dma_start(
                    out=pixcat[i][0:64, t0 : t0 + ts_, :],
                    in_=framesT[tilei, base : base + 64, t0 : t0 + ts_, :],
                )
                # shifted copy; last strip is one step short (frame T-1 has no
                # successor and t=T-1 of the curr-half is never read)
                te = min(t0 + ts_, T - 1)
                nc.gpsimd.dma_start(
                    out=pixcat[i][64:128, t0:te, :],
                    in_=framesT[tilei, base : base + 64, t0 + 1 : te + 1, :],
                )

        h0bf = consts.tile([128, 4, BC], bf16)
        nc.vector.memset(h0bf[:], 0.0)

        def enc_unit(beta, featbuf, i):
            pse = ps_ed.tile([128, BLK, BC], f32, name="ped")
            t0 = BLK * beta
            nc.tensor.matmul(
                out=pse[:],
                lhsT=wenc_sb[:],
                rhs=pixcat[i][:, t0 : t0 + BLK, :],
                start=True,
                stop=True,
            )
            # exact GELU via Erf: feat = (1 + erf((x+b)/sqrt2)) * (x+b), the
            # 0.5 is folded into w_ih host-side. erf and the bias-add both
            # read PSUM directly and run in parallel on ScE/DVE.
            xsb = encp.tile([128, BLK, BC], f32, name="xsb")
            nc.vector.tensor_scalar_add(out=xsb[:], in0=pse[:], scalar1=encb_sb[:, 0:1])
            erft = encp.tile([128, BLK, BC], f32, name="erft")
            nc.scalar.activation(
                out=erft[:],
                in_=pse[:],
                func=AF.Erf,
                bias=encb2_sb[:, 0:1],
                scale=0.7071067811865476,
            )
            nc.vector.scalar_tensor_tensor(
                out=featbuf[:, i, :, :],
                in0=erft[:],
                scalar=1.0,
                in1=xsb[:],
                op0=OP.add,
                op1=OP.mult,
            )

        def emit_enc(beta, featbuf):
            for i in range(4):
                enc_unit(beta, featbuf, i)

        def gx_unit(featbuf, gxbuf, m):
            psg = ps_gx.tile([128, BLK, BC], f32, name="psg")
            for k in range(4):
                nc.tensor.matmul(
                    out=psg[:],
                    lhsT=wih_sb[:, k, m, :],
                    rhs=featbuf[:, k, :, :],
                    start=(k == 0),
                    stop=(k == 3),
                )
            nc.scalar.activation(
                out=gxbuf[:, m, :, :],
                in_=psg[:],
                func=AF.Identity,
                bias=bcomb_sb[:, m : m + 1],
                scale=1.0,
            )

        def emit_gx(beta, featbuf, gxbuf):
            for m in range(12):
                gx_unit(featbuf, gxbuf, m)

        def emit_step(hbf, gxbuf, tt, outsbuf):
            # PSUM preloads (off the critical path): rz gets gx, n gets b_hh_n.
            # The 48 recurrent matmuls then accumulate straight on top
            # (start=False) and the activations read finished sums from PSUM.
            prz = ps_rz.tile([128, 8, BC], f32, name="prz")
            nc.scalar.activation(
                out=prz[:], in_=gxbuf[:, 0:8, tt, :], func=AF.Identity
            )
            pn = ps_n.tile([128, 4, BC], f32, name="pn")
            nc.vector.tensor_copy(out=pn[:], in_=bhhnb_sb[:])
            for m in range(4):
                for k in range(4):
                    nc.tensor.matmul(
                        out=prz[:, m, :],
                        lhsT=whh_sb[:, k, m, :],
                        rhs=hbf[:, k, :],
                        start=False,
                        stop=(k == 3),
                        skip_group_check=True,
                    )
            rz = stepp.tile([128, 8, BC], f32, name="rz")
            # r-sigmoid fires right after the r matmuls and hides under the
            # z/n matmul stream — it is off the step critical chain
            nc.scalar.activation(out=rz[:, 0:4, :], in_=prz[:, 0:4, :], func=AF.Sigmoid)
            for m in range(4, 8):
                for k in range(4):
                    nc.tensor.matmul(
                        out=prz[:, m, :],
                        lhsT=whh_sb[:, k, m, :],
                        rhs=hbf[:, k, :],
                        start=False,
                        stop=(k == 3),
                        skip_group_check=True,
                    )
            nc.scalar.activation(out=rz[:, 4:8, :], in_=prz[:, 4:8, :], func=AF.Sigmoid)
            # w = z*h and v = 1-z run on GpSimd (SBUF-only ops) under the
            # n-gate matmuls, keeping DVE free for the serial tail
            v = stepp.tile([128, 4, BC], f32, name="v")
            nc.gpsimd.tensor_scalar(
                out=v[:], in0=rz[:, 4:8, :], scalar1=-1.0, scalar2=1.0,
                op0=OP.mult, op1=OP.add,
            )
            w = stepp.tile([128, 4, BC], f32, name="w")
            nc.gpsimd.tensor_mul(out=w[:], in0=rz[:, 4:8, :], in1=hbf[:])
            for m in range(8, 12):
                for k in range(4):
                    nc.tensor.matmul(
                        out=pn[:, m - 8, :],
                        lhsT=whh_sb[:, k, m, :],
                        rhs=hbf[:, k, :],
                        start=False,
                        stop=(k == 3),
                        skip_group_check=True,
                    )
            # n = tanh(gx_n + r*(gh_n + b_hh_n)); pn already holds gh_n+b_hh_n.
            # Split in half so the serial tail after the last matmul is short:
            # the a-half runs while the b-half matmuls finish.
            for h2, (c0, c1) in enumerate(((0, 2), (2, 4))):
                t2 = stepp.tile([128, 2, BC], f32, name=f"t2{h2}")
                nc.vector.tensor_mul(
                    out=t2[:], in0=rz[:, c0:c1, :], in1=pn[:, c0:c1, :]
                )
                npre = stepp.tile([128, 2, BC], f32, name=f"npre{h2}")
                nc.vector.tensor_add(
                    out=npre[:], in0=t2[:], in1=gxbuf[:, 8 + c0 : 8 + c1, tt, :]
                )
                nsb = stepp.tile([128, 2, BC], f32, name=f"nsb{h2}")
                nc.scalar.activation(out=nsb[:], in_=npre[:], func=AF.Tanh)
                u = stepp.tile([128, 2, BC], f32, name=f"u{h2}")
                nc.vector.tensor_mul(out=u[:], in0=nsb[:], in1=v[:, c0:c1, :])
                nc.vector.tensor_add(
                    out=outsbuf[:, c0:c1, tt, :], in0=u[:], in1=w[:, c0:c1, :]
                )

        def dec_unit(beta, outsbuf, i2, currt):
            psd = ps_ed.tile([128, BLK, BC], f32, name="ped")
            for half in range(2):
                i = i2 * 2 + half
                nc.tensor.matmul(
                    out=psd[64 * half : 64 * half + 64, :, :],
                    lhsT=wdec_sb[:],
                    rhs=outsbuf[:, i, :, :],
                    start=True,
                    stop=True,
                )
            delta = decp.tile([128, BLK, BC], f32, name="delta")
            nc.scalar.activation(
                out=delta[:], in_=psd[:], func=AF.Tanh, bias=decb_sb[:, 0:1]
            )
            pred = decp.tile([128, BLK, BC], f32, name="pred")
            nc.vector.tensor_add(out=pred[:], in0=delta[:], in1=currt[:])
            nc.vector.tensor_scalar(
                out=pred[:],
                in0=pred[:],
                scalar1=0.0,
                scalar2=1.0,
                op0=OP.max,
                op1=OP.min,
            )
            nc.sync.dma_start(
                out=outT[i2, :, BLK * beta : BLK * beta + BLK, :],
                in_=pred[:],
            )

        def emit_pipeline():
            featbuf = featp.tile([128, 4, BLK, BC], bf16, name="featbuf")
            emit_enc(0, featbuf)
            gxbuf = gxp.tile([128, 12, BLK, BC], bf16, name="gxbuf")
            emit_gx(0, featbuf, gxbuf)

            hbf = h0bf[:]
            pending = []
            for beta in range(NB):
                cur_gx = gxbuf
                currts = []
                for i2 in range(2):
                    currt = decp.tile([128, BLK, BC], f32, name=f"curr{i2}")
                    nc.sync.dma_start(
                        out=currt[:],
                        in_=framesT[
                            i2, :, BLK * beta + 1 : BLK * beta + 1 + BLK, :
                        ],
                    )
                    currts.append(currt)
                # interleave the next block's enc/gx units (and the previous
                # block's decoder) between steps, ~3 units per step, instead
                # of letting 18 units pile up at the block boundary
                if beta + 1 < NB:
                    featbuf = featp.tile([128, 4, BLK, BC], bf16, name="featbuf")
                    gxbuf = gxp.tile([128, 12, BLK, BC], bf16, name="gxbuf")
                    fb, gb = featbuf, gxbuf
                    pending.extend(
                        [
                            lambda i=i, f=fb, b=beta + 1: enc_unit(b, f, i)
                            for i in range(4)
                        ]
                        + [lambda m=m, f=fb, g=gb: gx_unit(f, g, m) for m in range(12)]
                    )
                outsbuf = outsp.tile([128, 4, BLK, BC], bf16, name="outsbuf")
                for tt in range(BLK):
                    emit_step(hbf, cur_gx, tt, outsbuf)
                    hbf = outsbuf[:, :, tt, :]
                    for _ in range(3):
                        if pending:
                            pending.pop(0)()
                ob = outsbuf
                pending.extend(
                    [
                        lambda i2=i2, o=ob, c=currts[i2], b=beta: dec_unit(b, o, i2, c)
                        for i2 in range(2)
                    ]
                )
            while pending:
                pending.pop(0)()

        emit_pipeline()

    nc.compile()
    return nc


def _prep_weights(conv_w, conv_b, w_ih, w_hh, b_ih, b_hh, deconv_w, deconv_b, BC):
    """Host-side weight reshuffles into the kernel's permuted/tiled layouts."""
    import ml_dtypes

    bf = ml_dtypes.bfloat16

    idx = np.arange(HID)
    hmap = (idx % 32) * 16 + (idx // 32)  # h' -> h  (h' = ij*32 + c)

    # 0.5 from the erf-form GELU is folded into w_ih (feat' = 2*gelu(x))
    w_ih2 = 0.5 * w_ih.reshape(3, HID, HID)[:, hmap, :][:, :, hmap].reshape(
        3 * HID, HID
    )
    w_hh2 = w_hh.reshape(3, HID, HID)[:, hmap, :][:, :, hmap].reshape(3 * HID, HID)
    b_ih2 = b_ih.reshape(3, HID)[:, hmap].reshape(3 * HID)
    b_hh2 = b_hh.reshape(3, HID)[:, hmap].reshape(3 * HID)

    # (kk, k, m, mm): lhsT(k,m)[kk,mm] = W2[m*128+mm, k*128+kk]
    whh_t = np.ascontiguousarray(
        w_hh2.T.reshape(4, 128, 12, 128).transpose(1, 0, 2, 3)
    ).astype(bf)
    wih_t = np.ascontiguousarray(
        w_ih2.T.reshape(4, 128, 12, 128).transpose(1, 0, 2, 3)
    ).astype(bf)

    # encoder: rows u=16p+4j+q in [0,64) for the prev frame (c=1), rows
    # 64+u for the curr frame (c=0, via the pixcat +1 time shift);
    # cols j2*32+o. The block is identical for every patch-row i.
    wenc_h = np.zeros((128, 128), np.float32)
    u = np.arange(64)
    p, j, q = (u >> 4) & 3, (u >> 2) & 3, u & 3
    for s, c in ((0, 1), (1, 0)):
        blockw = np.zeros((64, 128), np.float32)
        for j2 in range(4):
            mask = j == j2
            blockw[mask, j2 * 32 : j2 * 32 + 32] = conv_w[:, c, p[mask], q[mask]].T
        wenc_h[64 * s : 64 * s + 64, :] = blockw
    wenc_t = wenc_h.astype(bf)

    # decoder: rows j*32+c, cols u2 = p*16 + j2*4 + q
    wdec_h = np.zeros((128, 64), np.float32)
    for jj in range(4):
        for c in range(CH):
            for pp in range(4):
                for qq in range(4):
                    wdec_h[jj * 32 + c, pp * 16 + jj * 4 + qq] = deconv_w[c, 0, pp, qq]
    wdec_t = wdec_h.astype(bf)

    encb_h = np.zeros((128, 1), np.float32)
    encb_h[:, 0] = conv_b[np.arange(128) % 32]
    encb2_h = encb_h * 0.7071067811865476

    bcomb_h = np.zeros((128, 12), np.float32)
    for m in range(12):
        g = m * 128 + np.arange(128)
        bcomb_h[:, m] = b_ih2[g] + (b_hh2[g] if m < 8 else 0.0)

    bhhnb_h = np.zeros((128, 4, BC), np.float32)
    for c in range(4):
        bhhnb_h[:, c, :] = b_hh2[1024 + c * 128 + np.arange(128)][:, None]

    decb_h = np.full((128, 1), float(deconv_b[0]), np.float32)

    return dict(
        whh=whh_t,
        wih=wih_t,
        wenc=wenc_t,
        wdec=wdec_t,
        encb=encb_h,
        encb2=encb2_h,
        bcomb=bcomb_h,
        bhhnb=bhhnb_h,
        decb=decb_h,
    )


def kernel(frames, conv_w, conv_b, w_ih, w_hh, b_ih, b_hh, deconv_w, deconv_b):
    from concourse.bass_utils import run_bass_kernel_spmd

    frames = np.asarray(frames, np.float32)
    conv_w = np.asarray(conv_w, np.float32)
    conv_b = np.asarray(conv_b, np.float32)
    w_ih = np.asarray(w_ih, np.float32)
    w_hh = np.asarray(w_hh, np.float32)
    b_ih = np.asarray(b_ih, np.float32)
    b_hh = np.asarray(b_hh, np.float32)
    deconv_w = np.asarray(deconv_w, np.float32)
    deconv_b = np.asarray(deconv_b, np.float32)

    B, T = frames.shape[0], frames.shape[1]
    T2 = T - 2
    BC = B // NCORES

    key = (T, BC)
    if key not in _cache:
        _cache[key] = _build(T, BC)
    nc = _cache[key]

    wmap = _prep_weights(
        conv_w, conv_b, w_ih, w_hh, b_ih, b_hh, deconv_w, deconv_b, BC
    )

    fr = frames.reshape(B, T, 256)
    in_maps = []
    for c in range(NCORES):
        sl = fr[c * BC : (c + 1) * BC]  # (BC, T, 256)
        framesT_c = np.ascontiguousarray(sl.transpose(2, 1, 0)).reshape(
            2, 128, T, BC
        )
        m = dict(wmap)
        m["framesT"] = framesT_c
        in_maps.append(m)

    res = run_bass_kernel_spmd(
        nc, in_maps, core_ids=list(range(NCORES)), trace=TRACE
    )
    global LAST_RESULT
    LAST_RESULT = res

    out = np.empty((B, T2, 1, 16, 16), np.float32)
    for c in range(NCORES):
        o = res.results[c]["outT"]  # (2, 128, T2, BC)
        o = o.reshape(256, T2, BC).transpose(2, 1, 0).reshape(BC, T2, 1, 16, 16)
        out[c * BC : (c + 1) * BC] = o
    return out
